# revision 18
# baseline (speedup 1.0000x reference)
"""Causal single-head attention (B=4, T=4096, C=1024, H=64) on 8 trn2 cores.

Sharding v2: 2 cores per batch, INTERLEAVED 512-query blocks. Core h of
batch b owns global query blocks {h, 2+h, 4+h, 6+h}. Slot s (s=0..3)
processes query block 2s+h against a key window of 8*(s+1) 128-key tiles
(80 tile-units/core vs 104 for the contiguous split). SPMD uniformity via
a 512-key host shift: core h=0's keys sit at buffer cols [512:4096)
(keys 0:3584 - block 6 needs no more), h=1 at [0:4096). Zero-padded tiles
are killed through a zeroed ones-column in v_all (no numerator or
denominator contribution), so one identical program runs on all cores.
Q-proj for slot s reads buffer cols [512(2s+1), +512) - same address on
both cores.

Perf structure:
  - matmul inputs bf16 (fp32 PSUM accumulation), exp fused with 1/sqrt(H)
  - projections col-packed 2x on array column halves; K^T/Q^T double-
    stacked on partition halves; scores row-packed 2x (tiles jA/jB on
    array row halves) into one [128,1024] psum, one exp (ACT) per pair
  - causal diagonal masked post-exp with GPSIMD affine_select (slot s
    diagonal = its last 4 tiles, uniform across cores)
  - O'^T accumulated with a fused ones-column denominator; normalization
    (divide by denominator) and transpose happen on the HOST
  - PE warmed up with dummy matmuls during the initial DMA wait; x loads
    split into 1MB chunks issued from 4 engine queues in parallel
"""

import numpy as np
import ml_dtypes

import concourse.bass as bass
import concourse.bacc as bacc
import concourse.tile as tile
from concourse import mybir
from concourse.bass_utils import run_bass_kernel_spmd

B, T, C, H = 4, 4096, 1024, 64
N_CORES = 8
NQB = 4              # query slots per core, 512 queries each
NCH = C // 128       # 8 contraction chunks
NKT = T // 128       # 32 key tiles
F32 = mybir.dt.float32
BF16 = mybir.dt.bfloat16

_nc_cache = {}


def build_module():
    if "nc" in _nc_cache:
        return _nc_cache["nc"]
    nc = bacc.Bacc("TRN2", target_bir_lowering=False, debug=False,
                   num_devices=N_CORES)
    xk = nc.dram_tensor("xk", [C, T], BF16, kind="ExternalInput").ap()
    wq = nc.dram_tensor("wq", [C, H], BF16, kind="ExternalInput").ap()
    wk = nc.dram_tensor("wk", [C, H], BF16, kind="ExternalInput").ap()
    wv = nc.dram_tensor("wv", [C, H], BF16, kind="ExternalInput").ap()
    ones_kv = nc.dram_tensor("ones_kv", [128, NKT], F32,
                             kind="ExternalInput").ap()
    ident2 = nc.dram_tensor("ident2", [128, 64], BF16,
                            kind="ExternalInput").ap()
    # [slot, H+1, 512]: O'^T plus denominator row; host normalizes and
    # transposes.
    out = nc.dram_tensor("out", [NQB, H + 1, 512], F32,
                         kind="ExternalOutput").ap()

    with tile.TileContext(nc) as tc:
        with (
            tc.tile_pool(name="consts", bufs=1) as consts,
            tc.tile_pool(name="xt", bufs=1) as xt_pool,
            tc.tile_pool(name="vtmp", bufs=2) as vtmp_pool,
            tc.tile_pool(name="exps", bufs=6) as exps_pool,
            tc.tile_pool(name="fin", bufs=2) as fin_pool,
            tc.tile_pool(name="ps_s", bufs=2, space="PSUM") as ps_s,
            tc.tile_pool(name="ps_o", bufs=2, space="PSUM") as ps_o,
            tc.tile_pool(name="ps_p", bufs=2, space="PSUM") as ps_p,
        ):
            # ---- PE warmup scratch (zeros; HAM needs ~3.4us of activity)
            warm_w = consts.tile([128, 64], BF16, name="warm_w")
            warm_x = consts.tile([128, 512], BF16, name="warm_x")
            nc.gpsimd.memset(warm_w[:], 0)
            nc.gpsimd.memset(warm_x[:], 0)

            # ---- constants / weights in SBUF ----
            # DMAs on one engine queue serialize end-to-end (next issue
            # waits on the previous transfer's semaphore), so each queue
            # carries its weights first, then x chunks in consumption
            # order. scalar/gpsimd queues stay short: their engines have
            # time-critical work (exp / affine_select) queued behind.
            w_sb = {}
            for name, ap, eng in (("wk", wk, nc.scalar), ("wq", wq, nc.gpsimd),
                                  ("wv", wv, nc.sync)):
                t = consts.tile([128, NCH, H], BF16, name=f"{name}_sb")
                eng.dma_start(t[:], ap.rearrange("(ch p) h -> p ch h", p=128))
                w_sb[name] = t
            id2_sb = consts.tile([128, 64], BF16, name="id2_sb")
            nc.sync.dma_start(id2_sb[:], ident2)
            ones_sb = consts.tile([128, NKT], F32, name="ones_sb")
            nc.sync.dma_start(ones_sb[:], ones_kv)

            # ---- persistent activations ----
            # kt2x: pair-group pg holds K^T for buffer t-blocks (2pg, 2pg+1)
            # on partition halves [0:64] / [64:128], columns pg*512 + w.
            kt2x = consts.tile([128, 2048], BF16, name="kt2x")
            # qt2x: Q^T for the 4 slots, duplicated on both partition halves.
            qt2x = consts.tile([128, 2048], BF16, name="qt2x")
            v_all = consts.tile([128, NKT, H + 1], BF16, name="v_all")

            nc.vector.tensor_copy(v_all[:, :, H], ones_sb[:])

            xk_r = xk.rearrange("(ch p) t -> p ch t", p=128)

            inv_sqrt_h = 1.0 / np.sqrt(np.float32(H))

            # ---- x loads: 1MB chunks spread across 4 engine queues ----
            xt_tiles = {}
            for tb0 in (0, 2, 4, 6):
                xt_tiles[tb0] = xt_pool.tile([128, NCH, 1024], BF16,
                                             tag=f"xt{tb0}", name=f"xt{tb0}")
            # xt0 split 3 ways so all queues push its 2MB at full HBM rate;
            # later tiles stream on sync alone (its only other work is the
            # final output stores).
            third = 1024 // 3 + 1  # 342
            for i, eng in enumerate((nc.scalar, nc.gpsimd, nc.sync)):
                c0 = i * third
                c1 = min(1024, (i + 1) * third)
                eng.dma_start(xt_tiles[0][:, :, c0:c1], xk_r[:, :, c0:c1])
            for tb0 in (2, 4, 6):
                for half in (1, 0):   # b-half first: Q(s) data before K(s)
                    c0 = tb0 * 512 + half * 512
                    nc.sync.dma_start(
                        xt_tiles[tb0][:, :, half * 512:(half + 1) * 512],
                        xk_r[:, :, c0:c0 + 512])

            # ---- PE warmup: dummy matmuls spanning the x-DMA wait so the
            # HAM clock gate is at 8/8 when real work starts (~7.5us cold).
            pw = ps_p.tile([64, 512], F32, tag="pp", name="pwarm")
            for _ in range(20):
                nc.tensor.matmul(pw[:], warm_w[:], warm_x[:],
                                 start=True, stop=True)

            def kt_slice(j):
                tb, s = j // 4, j % 4
                half, pg = tb % 2, tb // 2
                return kt2x[64 * half:64 * (half + 1),
                            pg * 512 + s * 128: pg * 512 + (s + 1) * 128]

            # ---- projection work as generators, pumped round-by-round
            # between attention items so PE work for upcoming stages fills
            # the slack while ACT chews exps, without ever blocking scores
            # for a full 2us proj call.
            proj_order = []
            proj_gens = {}

            def enqueue(tag, gen):
                proj_gens[tag] = gen
                proj_order.append(tag)

            def pump(n=1):
                while n > 0 and proj_order:
                    tag = proj_order[0]
                    try:
                        next(proj_gens[tag])
                        n -= 1
                    except StopIteration:
                        del proj_gens[tag]
                        proj_order.pop(0)

            def pump_until(tag):
                while tag in proj_order:
                    pump()

            def gen_rounds(wname_a, xa, wname_b, xb, name):
                pdst = ps_p.tile([128, 512], F32, tag="pp", name=name)
                for ch in range(NCH):
                    nc.tensor.matmul(pdst[0:64, :], w_sb[wname_a][:, ch, :],
                                     xa[:, ch, :],
                                     start=(ch == 0), stop=(ch == NCH - 1))
                    if xb is not None:
                        nc.tensor.matmul(pdst[64:128, :],
                                         w_sb[wname_b][:, ch, :],
                                         xb[:, ch, :],
                                         start=(ch == 0), stop=(ch == NCH - 1),
                                         tile_position=(0, 64))
                    yield
                return pdst

            def gen_projK(tb0):
                xt = xt_tiles[tb0]
                pk = yield from gen_rounds("wk", xt[:, :, 0:512],
                                           "wk", xt[:, :, 512:1024], f"pk{tb0}")
                pg = tb0 // 2
                nc.vector.tensor_copy(kt2x[:, pg * 512:(pg + 1) * 512], pk[:])

            def gen_projV(tb0):
                xt = xt_tiles[tb0]
                pv = yield from gen_rounds("wv", xt[:, :, 0:512],
                                           "wv", xt[:, :, 512:1024], f"pv{tb0}")
                vt = vtmp_pool.tile([128, 512], BF16, tag="vt", name=f"vt{tb0}")
                nc.vector.tensor_copy(vt[:], pv[:])
                yield
                for half in range(2):
                    for s in range(4):
                        j = 4 * (tb0 + half) + s
                        ptr = ps_p.tile([128, 64], BF16, tag="pp",
                                        name=f"ptr{j}")
                        nc.tensor.transpose(
                            ptr[:],
                            vt[64 * half:64 * (half + 1),
                               s * 128:(s + 1) * 128],
                            id2_sb[64 * half:64 * (half + 1), :])
                        nc.vector.tensor_copy(v_all[:, j, 0:H], ptr[:])
                        if s % 2 == 1:
                            yield

            def q_src(slot):
                # Q for slot s lives at buffer cols [512(2s+1), +512) =
                # second half of xt pair 2s.
                return xt_tiles[2 * slot][:, :, 512:1024]

            def gen_projQ(slot_a, slot_b):
                pq = yield from gen_rounds(
                    "wq", q_src(slot_a),
                    "wq", None if slot_b is None else q_src(slot_b),
                    f"pq{slot_a}")
                for half, slot in ((0, slot_a), (1, slot_b)):
                    if slot is None:
                        continue
                    sl = pq[64 * half:64 * (half + 1), :]
                    dst = slice(512 * slot, 512 * (slot + 1))
                    nc.vector.tensor_copy(qt2x[0:64, dst], sl)
                    nc.vector.tensor_copy(qt2x[64:128, dst], sl)

            attn_state = {}

            def attn_begin(qb):
                po = ps_o.tile([H + 1, 512], F32, tag="po", name=f"po{qb}")
                attn_state[qb] = dict(po=po, queue=[], jmax=8 * (qb + 1))

            def attn_flush_one(qb):
                st = attn_state[qb]
                js, es2 = st["queue"].pop(0)
                jmax = st["jmax"]
                for idx, j in enumerate(js):
                    nc.tensor.matmul(
                        st["po"][:], v_all[:, j, :],
                        es2[:, idx * 512:(idx + 1) * 512],
                        start=(j == 0), stop=(j == jmax - 1),
                        skip_group_check=True)

            def attn_items(qb, items, lag=2):
                # Items processed in blocks of two: both score pairs are
                # emitted back-to-back so the second pair's kt LDWEIGHTS
                # pull ahead under the first pair's streams (the O matmuls
                # use all 128 array rows and would block the pull-ahead),
                # then two deferred O pairs flush.
                st = attn_state[qb]
                jmax = st["jmax"]
                diag0 = jmax - 4
                qs_a = qt2x[0:64, qb * 512:(qb + 1) * 512]
                qs_b = qt2x[64:128, qb * 512:(qb + 1) * 512]
                for idx in range(0, len(items), 2):
                    for jA, jB in items[idx:idx + 2]:
                        ps = ps_s.tile([128, 1024], F32, tag="ps",
                                       name=f"s{qb}_{jA}")
                        nc.tensor.matmul(ps[:, 0:512], kt_slice(jA),
                                         qs_a, start=True, stop=True)
                        es2 = exps_pool.tile([128, 1024], BF16, tag="es",
                                             name=f"e{qb}_{jA}")
                        nc.tensor.matmul(ps[:, 512:1024], kt_slice(jB),
                                         qs_b, start=True, stop=True,
                                         tile_position=(64, 0))
                        nc.scalar.activation(
                            es2[:], ps[:],
                            mybir.ActivationFunctionType.Exp,
                            scale=float(inv_sqrt_h))
                        if jB >= diag0:
                            d = jB - diag0
                            nc.gpsimd.affine_select(
                                es2[:, 512:1024], es2[:, 512:1024],
                                pattern=[[1, 512]],
                                compare_op=mybir.AluOpType.is_ge,
                                fill=0.0, base=-(128 * d),
                                channel_multiplier=-1)
                        st["queue"].append(((jA, jB), es2))
                    while len(st["queue"]) > lag:
                        attn_flush_one(qb)
                    pump(1)

            def attn_flush(qb):
                st = attn_state[qb]
                while st["queue"]:
                    attn_flush_one(qb)

            def attn_final(qb):
                st = attn_state[qb]
                ot = fin_pool.tile([H + 1, 512], F32, tag="ot", name=f"ot{qb}")
                nc.vector.tensor_copy(ot[:], st["po"][:])
                nc.sync.dma_start(out[qb], ot[:])

            def G(g):
                return [(8 * g + s, 8 * g + 4 + s) for s in range(4)]

            # ---- interleaved emission schedule ----
            # Emission order must be topologically consistent: projK/projQ
            # fully pumped before the scores reading them, projV pumped
            # before the first O-matmul reading its v_all tiles (slot s's
            # group G(g) reads tiles [8g, 8g+8), so V(2s) is only needed
            # by G(s)). K/Q for the next slot are enqueued early and
            # drip-fed by the per-item pump.
            enqueue("k0", gen_projK(0))
            enqueue("q0", gen_projQ(0, None))
            pump_until("q0")
            attn_begin(0)
            attn_items(0, G(0)[:2])
            enqueue("v0", gen_projV(0))
            pump_until("v0")
            attn_items(0, G(0)[2:], lag=1)
            enqueue("q1", gen_projQ(1, None))
            pump_until("q1")
            attn_flush(0)
            attn_begin(1)
            enqueue("k2", gen_projK(2))
            enqueue("v2", gen_projV(2))
            attn_items(1, G(0))
            pump_until("v2")
            attn_final(0)
            attn_items(1, G(1), lag=1)
            enqueue("q23", gen_projQ(2, 3))
            pump_until("q23")
            attn_flush(1)
            attn_begin(2)
            enqueue("k4", gen_projK(4))
            enqueue("v4", gen_projV(4))
            attn_items(2, G(0))
            attn_items(2, G(1))
            pump_until("v4")
            attn_final(1)
            enqueue("k6", gen_projK(6))
            attn_items(2, G(2), lag=1)
            attn_flush(2)
            attn_begin(3)
            enqueue("v6", gen_projV(6))
            attn_items(3, G(0))
            attn_items(3, G(1))
            pump_until("v6")
            attn_final(2)
            attn_items(3, G(2))
            attn_items(3, G(3), lag=1)
            attn_flush(3)
            attn_final(3)
    nc.compile()
    _nc_cache["nc"] = nc
    return nc


def _core_inputs(x, Wq, Wk, Wv, core):
    b, h = core // 2, core % 2
    xkm = np.zeros((C, T), dtype=np.float32)
    pad = 512 * (1 - h)          # key shift: 512 for even cores, 0 for odd
    xkm[:, pad:] = x[b, 0:T - pad, :].T
    ones = np.zeros((128, NKT), dtype=np.float32)
    ones[:, pad // 128:] = 1.0
    id2 = np.zeros((128, 64), dtype=np.float32)
    id2[:64] = np.eye(64, dtype=np.float32)
    id2[64:] = np.eye(64, dtype=np.float32)
    bf = ml_dtypes.bfloat16
    return {
        "xk": np.ascontiguousarray(xkm.astype(bf)),
        "wq": np.ascontiguousarray(np.asarray(Wq, dtype=np.float32).astype(bf)),
        "wk": np.ascontiguousarray(np.asarray(Wk, dtype=np.float32).astype(bf)),
        "wv": np.ascontiguousarray(np.asarray(Wv, dtype=np.float32).astype(bf)),
        "ones_kv": ones,
        "ident2": id2.astype(bf),
    }


def kernel(x, Wq, Wk, Wv):
    x = np.asarray(x, dtype=np.float32)
    nc = build_module()
    in_maps = [_core_inputs(x, Wq, Wk, Wv, c) for c in range(N_CORES)]
    res = run_bass_kernel_spmd(nc, in_maps, core_ids=list(range(N_CORES)))
    out = np.empty((B, T, H), dtype=np.float32)
    for core in range(N_CORES):
        b, h = core // 2, core % 2
        arr = res.results[core]["out"]          # [slot, H+1, 512]
        for s in range(NQB):
            blk = arr[s]
            r0 = 512 * (2 * s + h)
            out[b, r0:r0 + 512, :] = (blk[:H] / blk[H:H + 1]).T
    return out
